# revision 18
# baseline (speedup 1.0000x reference)
"""VQ codebook lookup kernel for Trainium2 (8 NeuronCores, data-parallel).

Computes out[b] = values[argmin_k ||x[b] - keys[k]||] for
x [65536, 512], keys/values [1024, 512] fp32.

Strategy (per core, batch shard of 8192 rows):
  - argmin of distance == argmax of s = 2*x.k - |k|^2 (sqrt and the
    |x|^2 row offset do not change the argmin).
  - near-fp32 matmul precision at better-than-bf16 cost: one fp16
    main pass (x and 2k^T in fp16, 2^-11 rel precision) plus one fp8
    e4m3 DoubleRow correction pass (2x rate) carrying the two cross
    residual terms  x_res . k16  and  x16 . k_res.
  - fp8 cannot represent the 2^-11-scale residuals at true scale, so
    residual operands are pre-scaled by 2^11 on the host and the fp16
    main operands are scaled by 2^6 / 2^5; every PSUM contribution
    then lands at 2^11x true scale and a single accumulation group
    works.  argmax is scale-invariant; we subtract 2^11*|k|^2.
  - Device per 128-row tile: 8 fp16 matmuls (N=512) + 4 fp8 DoubleRow
    matmuls (256-deep contraction each, N=256) per N-quarter -> DVE
    subtract of scaled |k|^2 fused with the PSUM->SBUF move -> DVE
    MAX8/FIND_INDEX8 per-row argmax -> indirect-DMA gather of values
    rows -> DMA out.
"""

import numpy as np

_B = 65536
_D = 512
_K = 1024
_NCORES = 8
_BL = _B // _NCORES  # 8192 rows per core
_P = 128
_BBLK = 512          # b columns loaded per DMA
_BT = 128            # b rows per matmul tile (PSUM partition dim)
_DC = _D // _P       # 4 contraction chunks

_cached = None


def _build():
    import concourse.mybir as mybir
    from concourse import bacc
    from concourse.bass import IndirectOffsetOnAxis
    from concourse.tile import TileContext

    f32 = mybir.dt.float32
    f16 = mybir.dt.float16
    f8 = mybir.dt.float8e4
    u32 = mybir.dt.uint32
    DR = mybir.MatmulPerfMode.DoubleRow

    nc = bacc.Bacc("TRN2", target_bir_lowering=False, debug=False,
                   num_devices=_NCORES)
    nblk = _BL // _BBLK
    # x main operand: fp16(x)*2^6, block-major so each partition reads one
    # contiguous 4KB line per block (small per-partition chunks gate the
    # x-in DMA queue at ~66GB/s and starve the PE).
    x16 = nc.dram_tensor("x16", [nblk, _P, _DC, _BBLK], f16,
                         kind="ExternalInput")
    # x fp8 operands: u=0..3 -> e4m3((x - fp16(x))*2^11), u=4..7 -> e4m3(x)
    x8 = nc.dram_tensor("x8", [nblk, _P, 2 * _DC, _BBLK], f8,
                        kind="ExternalInput")
    # keys main operand: fp16(2k^T)*2^5, [512, 1024]
    k16 = nc.dram_tensor("k16", [_D, _K], f16, kind="ExternalInput")
    # keys fp8: rows 0..511 = e4m3(2k^T), rows 512..1023 =
    #           e4m3((2k^T - fp16(2k^T))*2^11)
    k8 = nc.dram_tensor("k8", [2 * _D, _K], f8, kind="ExternalInput")
    k2r = nc.dram_tensor("k2r", [_P, _K], f32, kind="ExternalInput")
    vals = nc.dram_tensor("vals", [_K, _D], f32, kind="ExternalInput")
    out = nc.dram_tensor("out", [_BL, _D], f32, kind="ExternalOutput")

    k16v = k16.rearrange("(dc p) k -> p dc k", p=_P)   # [128, 4, 1024]
    k8v = k8.rearrange("(u p) k -> p u k", p=_P)       # [128, 8, 1024]

    with TileContext(nc) as tc:
        with (
            tc.tile_pool(name="const", bufs=1) as cpool,
            tc.tile_pool(name="xp", bufs=3) as xpool,
            tc.tile_pool(name="warm", bufs=1) as warmpool,
            tc.tile_pool(name="sp", bufs=3) as spool,
            tc.tile_pool(name="s0p", bufs=3) as s0pool,
            tc.tile_pool(name="st", bufs=4) as stpool,
            tc.tile_pool(name="gp", bufs=4) as gpool,
            tc.tile_pool(name="ps", bufs=3, space="PSUM") as pspool,
            tc.tile_pool(name="wps", bufs=1, space="PSUM") as wpspool,
        ):
            # Const loads on the Scalar engine's HWDGE queue so they overlap
            # with the x-block loads issued from the Sync engine.  Ordered by
            # when tile 0 consumes them: k16 first (fp16 main pass runs first).
            k16_sb = cpool.tile([_P, _DC, _K], f16)
            k8_sb = cpool.tile([_P, 2 * _DC, _K], f8)
            k2_sb = cpool.tile([_P, _K], f32)
            nc.scalar.dma_start(k16_sb[:, :, 0:512], k16v[:, :, 0:512])
            nc.scalar.dma_start(k16_sb[:, :, 512:1024], k16v[:, :, 512:1024])
            nc.gpsimd.dma_start(k8_sb[:], k8v[:, :, :])
            nc.gpsimd.dma_start(k2_sb[:], k2r[:, :])

            # Pre-warm the PE clock (HAM) during the initial DMA wait:
            # ~4us of dummy matmuls on memset scratch lifts the PE from
            # 1.2GHz to 2.4GHz before the real stream begins.
            wsrc = warmpool.tile([_P, 64], f16)
            nc.vector.memset(wsrc[:], 0.0)
            wps = wpspool.tile([_P, 64], f32)
            for _ in range(72):
                nc.tensor.matmul(wps[:64, :], lhsT=wsrc[:, :64], rhs=wsrc[:],
                                 start=True, stop=True)

            for bi in range(nblk):
                xt16 = xpool.tile([_P, _DC, _BBLK], f16, tag="xt16")
                xt8 = xpool.tile([_P, 2 * _DC, _BBLK], f8, tag="xt8")
                if bi == 0:
                    # Split block 0 so the first b-tile lands (and the PE
                    # starts) before the rest of the block arrives.
                    nc.sync.dma_start(xt16[:, :, 0:_BT], x16[0, :, :, 0:_BT])
                    nc.sync.dma_start(xt8[:, :, 0:_BT], x8[0, :, :, 0:_BT])
                    nc.sync.dma_start(xt16[:, :, _BT:], x16[0, :, :, _BT:])
                    nc.sync.dma_start(xt8[:, :, _BT:], x8[0, :, :, _BT:])
                else:
                    nc.sync.dma_start(xt16[:], x16[bi, :, :, :])
                    nc.sync.dma_start(xt8[:], x8[bi, :, :, :])

                for sub in range(_BBLK // _BT):
                    bt = bi * (_BBLK // _BT) + sub
                    bsl = slice(sub * _BT, (sub + 1) * _BT)
                    ps = pspool.tile([_P, _K], f32)
                    s0 = s0pool.tile([_P, _K], f32)
                    s = spool.tile([_P, _K], f32)
                    # fp16 main pass: psum = 2^11 * x.(2k)
                    for h in range(2):
                        hsl = slice(h * 512, (h + 1) * 512)
                        for dc in range(_DC):
                            nc.tensor.matmul(ps[:, hsl],
                                             lhsT=xt16[:, dc, bsl],
                                             rhs=k16_sb[:, dc, hsl],
                                             start=(dc == 0), stop=False)
                    # fp8 DoubleRow corrections: x_res.k16 + x16.k_res,
                    # contraction 1024 as 4 double-subtile matmuls per
                    # 256-wide N quarter.
                    for h in range(2):
                        for q in (2 * h, 2 * h + 1):
                            qsl = slice(q * 256, (q + 1) * 256)
                            for j in range(4):
                                nc.tensor.matmul(ps[:, qsl],
                                                 lhsT=xt8[:, 2 * j:2 * j + 2, bsl],
                                                 rhs=k8_sb[:, 2 * j:2 * j + 2, qsl],
                                                 perf_mode=DR,
                                                 start=False, stop=(j == 3))
                        # s = 2^11*(2x.k - |k|^2).  Balance the PSUM->SBUF
                        # subtract across engines: half 0 on DVE (which can
                        # read PSUM directly), half 1 via ACT copy + Pool sub
                        # (Pool cannot read PSUM on TRN2).
                        hsl = slice(h * 512, (h + 1) * 512)
                        if h == 0:
                            nc.vector.tensor_sub(
                                out=s[:, hsl], in0=ps[:, hsl], in1=k2_sb[:, hsl])
                        else:
                            nc.scalar.copy(out=s0[:, hsl], in_=ps[:, hsl])
                            nc.gpsimd.tensor_sub(
                                out=s[:, hsl], in0=s0[:, hsl], in1=k2_sb[:, hsl])
                    mx = stpool.tile([_P, 8], f32)
                    nc.vector.max(out=mx[:], in_=s[:])
                    idx = stpool.tile([_P, 8], u32)
                    nc.vector.max_index(out=idx[:], in_max=mx[:], in_values=s[:])

                    g = gpool.tile([_P, _D], f32)
                    nc.gpsimd.indirect_dma_start(
                        out=g[:],
                        out_offset=None,
                        in_=vals[:, :],
                        in_offset=IndirectOffsetOnAxis(ap=idx[:, :1], axis=0),
                    )
                    nc.scalar.dma_start(out[bt * _BT:(bt + 1) * _BT, :], g[:])

    nc.compile()
    return nc


def _get_nc():
    global _cached
    if _cached is None:
        _cached = _build()
    return _cached


def _prepare_in_maps(x, keys, values):
    import ml_dtypes

    f8 = ml_dtypes.float8_e4m3

    x = np.asarray(x, dtype=np.float32)
    keys = np.asarray(keys, dtype=np.float32)
    values = np.asarray(values, dtype=np.float32)

    kT = np.ascontiguousarray((2.0 * keys).T)            # [512, 1024] f32
    k1_16 = kT.astype(np.float16)
    k16m = (k1_16 * np.float16(32)).astype(np.float16)   # 2^5 scale, exact
    kres = (kT - k1_16.astype(np.float32)) * 2048.0      # 2^11 scale
    k8m = np.concatenate([kT.astype(f8), kres.astype(f8)], axis=0)

    k2 = np.einsum("kd,kd->k", keys, keys).astype(np.float32) * 2048.0
    k2r = np.ascontiguousarray(np.broadcast_to(k2, (_P, _K)))

    nblk = _BL // _BBLK

    def _blocked(a, u):
        # [u*128, 8192] -> [nblk, 128, u, 512] (contiguous per partition/block)
        return np.ascontiguousarray(
            a.reshape(u, _P, nblk, _BBLK).transpose(2, 1, 0, 3))

    in_maps = []
    for c in range(_NCORES):
        xs = np.ascontiguousarray(x[c * _BL:(c + 1) * _BL].T)  # [512, 8192]
        x1_16 = xs.astype(np.float16)
        x16m = _blocked((x1_16 * np.float16(64)).astype(np.float16), _DC)
        xres = (xs - x1_16.astype(np.float32)) * 2048.0
        x8m = _blocked(
            np.concatenate([xres.astype(f8), xs.astype(f8)], axis=0), 2 * _DC)
        in_maps.append({"x16": x16m, "x8": x8m, "k16": k16m,
                        "k8": k8m, "k2r": k2r, "vals": values})
    return in_maps


def kernel(x, keys, values):
    from concourse.bass_utils import run_bass_kernel_spmd

    nc = _get_nc()
    in_maps = _prepare_in_maps(x, keys, values)
    res = run_bass_kernel_spmd(nc, in_maps, core_ids=list(range(_NCORES)))
    return np.concatenate([r["out"] for r in res.results], axis=0)


# revision 23
# speedup vs baseline: 1.0308x; 1.0308x over previous
"""VQ codebook lookup kernel for Trainium2 (8 NeuronCores, data-parallel).

Computes out[b] = values[argmin_k ||x[b] - keys[k]||] for
x [65536, 512], keys/values [1024, 512] fp32.

Strategy (per core, batch shard of 8192 rows):
  - argmin of distance == argmax of s = 2*x.k - |k|^2 (sqrt and the
    |x|^2 row offset do not change the argmin).
  - near-fp32 matmul precision at better-than-bf16 cost: one fp16
    main pass (x and 2k^T in fp16, 2^-11 rel precision) plus one fp8
    e4m3 DoubleRow correction pass (2x rate) carrying the two cross
    residual terms  x_res . k16  and  x16 . k_res.
  - fp8 cannot represent the 2^-11-scale residuals at true scale, so
    residual operands are pre-scaled by 2^11 on the host and the fp16
    main operands are scaled by 2^6 / 2^5; every PSUM contribution
    then lands at 2^11x true scale and a single accumulation group
    works.  argmax is scale-invariant; we subtract 2^11*|k|^2.
  - Device per 128-row tile: 8 fp16 matmuls (N=512) + 4 fp8 DoubleRow
    matmuls (256-deep contraction each, N=256) per N-quarter -> DVE
    subtract of scaled |k|^2 fused with the PSUM->SBUF move -> DVE
    MAX8/FIND_INDEX8 per-row argmax -> indirect-DMA gather of values
    rows -> DMA out.
"""

import numpy as np

_B = 65536
_D = 512
_K = 1024
_NCORES = 8
_BL = _B // _NCORES  # 8192 rows per core
_P = 128
_BBLK = 512          # b columns loaded per DMA
_BT = 128            # b rows per matmul tile (PSUM partition dim)
_DC = _D // _P       # 4 contraction chunks

_cached = None


def _build():
    import concourse.mybir as mybir
    from concourse import bacc
    from concourse.bass import IndirectOffsetOnAxis
    from concourse.tile import TileContext

    f32 = mybir.dt.float32
    f16 = mybir.dt.float16
    f8 = mybir.dt.float8e4
    u32 = mybir.dt.uint32
    DR = mybir.MatmulPerfMode.DoubleRow

    nc = bacc.Bacc("TRN2", target_bir_lowering=False, debug=False,
                   num_devices=_NCORES)
    nblk = _BL // _BBLK
    # x main operand: fp16(x)*2^6, block-major so each partition reads one
    # contiguous 4KB line per block (small per-partition chunks gate the
    # x-in DMA queue at ~66GB/s and starve the PE).
    x16 = nc.dram_tensor("x16", [nblk, _P, _DC, _BBLK], f16,
                         kind="ExternalInput")
    # x fp8 operands: u=0..3 -> e4m3((x - fp16(x))*2^11), u=4..7 -> e4m3(x)
    x8 = nc.dram_tensor("x8", [nblk, _P, 2 * _DC, _BBLK], f8,
                        kind="ExternalInput")
    # keys main operand: fp16(2k^T)*2^5, [512, 1024]
    k16 = nc.dram_tensor("k16", [_D, _K], f16, kind="ExternalInput")
    # keys fp8: rows 0..511 = e4m3(2k^T), rows 512..1023 =
    #           e4m3((2k^T - fp16(2k^T))*2^11)
    k8 = nc.dram_tensor("k8", [2 * _D, _K], f8, kind="ExternalInput")
    k2r = nc.dram_tensor("k2r", [_P, _K], f32, kind="ExternalInput")
    vals = nc.dram_tensor("vals", [_K, _D], f32, kind="ExternalInput")
    out = nc.dram_tensor("out", [_BL, _D], f32, kind="ExternalOutput")

    k16v = k16.rearrange("(dc p) k -> p dc k", p=_P)   # [128, 4, 1024]
    k8v = k8.rearrange("(u p) k -> p u k", p=_P)       # [128, 8, 1024]

    with TileContext(nc) as tc:
        with (
            tc.tile_pool(name="const", bufs=1) as cpool,
            tc.tile_pool(name="xp", bufs=3) as xpool,
            tc.tile_pool(name="warm", bufs=1) as warmpool,
            tc.tile_pool(name="sp", bufs=3) as spool,
            tc.tile_pool(name="st", bufs=4) as stpool,
            tc.tile_pool(name="gp", bufs=4) as gpool,
            tc.tile_pool(name="ps", bufs=3, space="PSUM") as pspool,
            tc.tile_pool(name="wps", bufs=1, space="PSUM") as wpspool,
        ):
            # Const loads on the Scalar engine's HWDGE queue so they overlap
            # with the x-block loads issued from the Sync engine.  Ordered by
            # when tile 0 consumes them: k16 first (fp16 main pass runs first).
            k16_sb = cpool.tile([_P, _DC, _K], f16)
            k8_sb = cpool.tile([_P, 2 * _DC, _K], f8)
            k2_sb = cpool.tile([_P, _K], f32)
            nc.scalar.dma_start(k16_sb[:, :, 0:512], k16v[:, :, 0:512])
            nc.scalar.dma_start(k16_sb[:, :, 512:1024], k16v[:, :, 512:1024])
            nc.gpsimd.dma_start(k8_sb[:], k8v[:, :, :])
            nc.gpsimd.dma_start(k2_sb[:], k2r[:, :])

            # Pre-warm the PE clock (HAM) during the initial DMA wait:
            # ~4us of dummy matmuls on memset scratch lifts the PE from
            # 1.2GHz to 2.4GHz before the real stream begins.
            wsrc = warmpool.tile([_P, 64], f16)
            nc.vector.memset(wsrc[:], 0.0)
            wps = wpspool.tile([_P, 64], f32)
            for _ in range(72):
                nc.tensor.matmul(wps[:64, :], lhsT=wsrc[:, :64], rhs=wsrc[:],
                                 start=True, stop=True)

            pend_out = []
            for bi in range(nblk):
                xt16 = xpool.tile([_P, _DC, _BBLK], f16, tag="xt16")
                xt8 = xpool.tile([_P, 2 * _DC, _BBLK], f8, tag="xt8")
                if bi == 0:
                    # Split block 0 so the first b-tile lands (and the PE
                    # starts) before the rest of the block arrives.
                    nc.sync.dma_start(xt16[:, :, 0:_BT], x16[0, :, :, 0:_BT])
                    nc.sync.dma_start(xt8[:, :, 0:_BT], x8[0, :, :, 0:_BT])
                    nc.sync.dma_start(xt16[:, :, _BT:], x16[0, :, :, _BT:])
                    nc.sync.dma_start(xt8[:, :, _BT:], x8[0, :, :, _BT:])
                else:
                    nc.sync.dma_start(xt16[:], x16[bi, :, :, :])
                    nc.sync.dma_start(xt8[:], x8[bi, :, :, :])

                for sub in range(_BBLK // _BT):
                    bt = bi * (_BBLK // _BT) + sub
                    bsl = slice(sub * _BT, (sub + 1) * _BT)
                    ps = pspool.tile([_P, _K], f32)
                    s = spool.tile([_P, _K], f32)
                    # fp16 main pass: psum = 2^11 * x.(2k)
                    for h in range(2):
                        hsl = slice(h * 512, (h + 1) * 512)
                        for dc in range(_DC):
                            nc.tensor.matmul(ps[:, hsl],
                                             lhsT=xt16[:, dc, bsl],
                                             rhs=k16_sb[:, dc, hsl],
                                             start=(dc == 0), stop=False)
                    # fp8 DoubleRow corrections: x_res.k16 + x16.k_res,
                    # contraction 1024 as 4 double-subtile matmuls per
                    # 256-wide N quarter.
                    for h in range(2):
                        for q in (2 * h, 2 * h + 1):
                            qsl = slice(q * 256, (q + 1) * 256)
                            for j in range(4):
                                nc.tensor.matmul(ps[:, qsl],
                                                 lhsT=xt8[:, 2 * j:2 * j + 2, bsl],
                                                 rhs=k8_sb[:, 2 * j:2 * j + 2, qsl],
                                                 perf_mode=DR,
                                                 start=False, stop=(j == 3))
                        # s = 2^11*(2x.k - |k|^2), fused PSUM->SBUF move
                        # (DVE is the only engine that can read PSUM and do a
                        # free-axis tensor-tensor subtract on TRN2)
                        hsl = slice(h * 512, (h + 1) * 512)
                        nc.vector.tensor_sub(
                            out=s[:, hsl], in0=ps[:, hsl], in1=k2_sb[:, hsl])
                    mx = stpool.tile([_P, 8], f32)
                    nc.vector.max(out=mx[:], in_=s[:])
                    idx = stpool.tile([_P, 8], u32)
                    nc.vector.max_index(out=idx[:], in_max=mx[:], in_values=s[:])

                    g = gpool.tile([_P, _D], f32)
                    nc.gpsimd.indirect_dma_start(
                        out=g[:],
                        out_offset=None,
                        in_=vals[:, :],
                        in_offset=IndirectOffsetOnAxis(ap=idx[:, :1], axis=0),
                    )
                    # Lag the out-DMA by one tile: its engine-blocking wait on
                    # the gather semaphore must not delay queue-mates that
                    # free earlier pipeline resources.
                    pend_out.append((bt, g))
                    if len(pend_out) > 1:
                        pbt, pg = pend_out.pop(0)
                        nc.scalar.dma_start(
                            out[pbt * _BT:(pbt + 1) * _BT, :], pg[:])
            for pbt, pg in pend_out:
                nc.scalar.dma_start(out[pbt * _BT:(pbt + 1) * _BT, :], pg[:])

    nc.compile()
    return nc


def _get_nc():
    global _cached
    if _cached is None:
        _cached = _build()
    return _cached


def _prepare_in_maps(x, keys, values):
    import ml_dtypes

    f8 = ml_dtypes.float8_e4m3

    x = np.asarray(x, dtype=np.float32)
    keys = np.asarray(keys, dtype=np.float32)
    values = np.asarray(values, dtype=np.float32)

    kT = np.ascontiguousarray((2.0 * keys).T)            # [512, 1024] f32
    k1_16 = kT.astype(np.float16)
    k16m = (k1_16 * np.float16(32)).astype(np.float16)   # 2^5 scale, exact
    kres = (kT - k1_16.astype(np.float32)) * 2048.0      # 2^11 scale
    k8m = np.concatenate([kT.astype(f8), kres.astype(f8)], axis=0)

    k2 = np.einsum("kd,kd->k", keys, keys).astype(np.float32) * 2048.0
    k2r = np.ascontiguousarray(np.broadcast_to(k2, (_P, _K)))

    nblk = _BL // _BBLK

    def _blocked(a, u):
        # [u*128, 8192] -> [nblk, 128, u, 512] (contiguous per partition/block)
        return np.ascontiguousarray(
            a.reshape(u, _P, nblk, _BBLK).transpose(2, 1, 0, 3))

    in_maps = []
    for c in range(_NCORES):
        xs = np.ascontiguousarray(x[c * _BL:(c + 1) * _BL].T)  # [512, 8192]
        x1_16 = xs.astype(np.float16)
        x16m = _blocked((x1_16 * np.float16(64)).astype(np.float16), _DC)
        xres = (xs - x1_16.astype(np.float32)) * 2048.0
        x8m = _blocked(
            np.concatenate([xres.astype(f8), xs.astype(f8)], axis=0), 2 * _DC)
        in_maps.append({"x16": x16m, "x8": x8m, "k16": k16m,
                        "k8": k8m, "k2r": k2r, "vals": values})
    return in_maps


def kernel(x, keys, values):
    from concourse.bass_utils import run_bass_kernel_spmd

    nc = _get_nc()
    in_maps = _prepare_in_maps(x, keys, values)
    res = run_bass_kernel_spmd(nc, in_maps, core_ids=list(range(_NCORES)))
    return np.concatenate([r["out"] for r in res.results], axis=0)
